# revision 10
# baseline (speedup 1.0000x reference)
"""Trainium2 Bass kernel for the 4-layer dense transformer (nn_BDH_GPU_65326452572468).

Sharding: 8 cores = 4 batches x 2 head-pairs. Core c handles batch c//2 and
heads {0,1} (c even) or {2,3} (c odd). Per layer, each core computes its two
heads' attention and dv contribution; dv is all-reduced within the core pair,
after which v stays replicated. Logits are taken from the even core of each pair.

v2 design notes (vs the v1 baseline):
- v lives TRANSPOSED on device: vT [d=128 partitions, T free]. LN stats are
  computed with ones-weight matmuls over the partition (d) axis; the per-token
  affine is applied via gpsimd partition_broadcast + two DVE ops.
- Attention computes scores TRANSPOSED (S^T[s,t] blocks) by swapping matmul
  operands, so P^T is produced directly and the ~1100 PE transposes of the v1
  kernel disappear. Softmax row-sums come free from a ones-column appended to
  the V operand (vnh_aug [s, 65]); normalization scales a^T per-column after
  accumulation (relu commutes with the positive scale).
- Causal masking: diagonal s-blocks stream only their valid t-range
  (N in {512,384,256,128}) and the one triangular 128-wide subtile is masked
  by multiplying exp by a 0/1 triu mask (bf16, cheap).
- x = relu(vn@dx) / yi-gemms use tile_position row-packing (K=32 -> 4-way /
  2-way concurrent in the 128x128 PE array).
- All matmuls bf16 with fp32 PSUM; softmax uses the constant-bias trick
  (scores bounded ~12.2, bias 16).
"""
import sys
import numpy as np

sys.path.insert(0, "/opt/trn_rl_repo")

import ml_dtypes

import concourse.bass as bass
import concourse.mybir as mybir
import concourse.tile as tile
from concourse import bacc
from concourse.bass_utils import run_bass_kernel_spmd

BF = ml_dtypes.bfloat16
FP32 = mybir.dt.float32
BF16 = mybir.dt.bfloat16
AL = mybir.AluOpType
AF = mybir.ActivationFunctionType
AX = mybir.AxisListType

D = 128
H = 4
L = 4
N = 4096
VOCAB = 256
DH = 32          # D // H
NH = 1024        # N // H
EPS = 1e-5
M_BIAS = 16.0    # constant softmax shift (max observed score ~12.2)
NCORES = 8
NCH = NH // 128  # 8 i-chunks per head
RSQRT_D = float(1.0 / np.sqrt(128.0))


def _blob_offsets(T, apply_g1b1, apply_g2b2):
    """Word offsets (per 128-partition row) of each packed constant."""
    offs, o = {}, 0
    def add(name, words):
        nonlocal o
        offs[name] = (o, words)
        o += words
    add("v0T", T // 2)               # bf16 [128, T]  (d-major v0)
    add("dxl2", NH // 2)             # bf16 [128, NH]: h0,h1,h0,h1 in 32-row groups
    add("dyl", NH // 2)              # bf16 rows 32-63 h0, 64-95 h1
    add("encl", NCH * D)             # bf16 [128, 2*NCH*D]
    add("trig", 4 * (2 * (T // 64) + 128))  # fp32 rope seed tables
    add("triu", 64)                  # bf16 [128,128] 0/1, keep t>=s
    add("ident", 64)                 # bf16 [128,128]
    add("rwt", VOCAB // 2)           # bf16 [128, VOCAB]
    add("sel", 64)                   # bf16 [128,128] d -> compact-row selection
    if apply_g1b1:
        add("g1c", 1); add("b1c", 1)
    if apply_g2b2:
        add("g2f", 1); add("b2f", 1)
    offs["_total"] = o
    return offs


def build_kernel(T: int, apply_g1b1: bool, apply_g2b2: bool,
                 use_collective: bool = True, n_layers: int = L):
    NT = T // 128
    NM = T // 512
    nc = bacc.Bacc("TRN2", target_bir_lowering=False, debug=False,
                   num_devices=NCORES)

    offs = _blob_offsets(T, apply_g1b1, apply_g2b2)
    blob_e = nc.dram_tensor("blob", [128, offs["_total"]], FP32,
                            kind="ExternalInput").ap()
    logits_e = nc.dram_tensor("logits", [T, VOCAB], FP32, kind="ExternalOutput").ap()

    def bslice(name, rows=128):
        o, w = offs[name]
        return blob_e[0:rows, o:o + w]

    from contextlib import ExitStack
    with tile.TileContext(nc) as tc, ExitStack() as stack:
        persist = stack.enter_context(tc.tile_pool(name="persist", bufs=1))
        work = stack.enter_context(tc.tile_pool(name="work", bufs=3))
        ropet = stack.enter_context(tc.tile_pool(name="ropet", bufs=2))
        stat = stack.enter_context(tc.tile_pool(name="stat", bufs=2))
        dramp = stack.enter_context(tc.tile_pool(name="dramp", bufs=2, space="DRAM"))
        ps_s = stack.enter_context(tc.tile_pool(name="ps_s", bufs=2, space="PSUM"))
        ps_a = stack.enter_context(tc.tile_pool(name="ps_a", bufs=2, space="PSUM"))
        ps_w = stack.enter_context(tc.tile_pool(name="ps_w", bufs=3, space="PSUM"))
        ps_t = stack.enter_context(tc.tile_pool(name="ps_t", bufs=1, space="PSUM"))

        # ---- persistent SBUF state ----
        vT = persist.tile([128, T], FP32)             # v transposed [d, t]
        dv_sb = persist.tile([128, T], FP32)          # dv^T (also rope scratch)
        dv2_sb = persist.tile([128, T], FP32)         # reduced dv^T (also rope scratch)
        vb_sb = persist.tile([128, T], BF16)          # bf16 cast of vT
        sq_sb = persist.tile([128, T], BF16)          # (vT^2)/128 bf16
        vnTc = persist.tile([128, T], BF16)           # compact ln1(v): h0,h1,h0,h1
        vnh = persist.tile([128, NT * 96], BF16)      # [s,96]: col0=ones, 32-95=vn
        aT_sb = persist.tile([96, T], BF16)           # a^T rows 32-63 h0, 64-95 h1
        x_sb = persist.tile([128, NCH * T], BF16)     # x (one head), chunk c at c*T
        xr_sb = persist.tile([128, NCH * T], BF16)    # rope(x); reused as yi
        cos_sb = persist.tile([128, 4 * T], BF16)
        sin_sb = persist.tile([128, 4 * T], BF16)
        dxl2_sb = persist.tile([128, NH], BF16)
        dyl_sb = persist.tile([96, NH], BF16)         # rows 32-63 h0, 64-95 h1
        encl_sb = persist.tile([128, 2 * NCH * D], BF16)
        sel_sb = persist.tile([128, 128], BF16)
        triu_sb = persist.tile([128, 128], BF16)
        id_sb = persist.tile([128, 128], BF16)
        rwt_sb = persist.tile([D, VOCAB], BF16)
        mbias_sb = persist.tile([128, 1], FP32)
        eps_sb = persist.tile([128, 1], FP32)
        onesA = persist.tile([128, 1], BF16)          # 1/128
        onesB = persist.tile([128, 1], BF16)          # 1.0

        nc.sync.dma_start(vb_sb[:], bslice("v0T").bitcast(BF16))
        nc.vector.tensor_copy(vT[:], vb_sb[:])
        nc.sync.dma_start(dxl2_sb[:], bslice("dxl2").bitcast(BF16))
        nc.sync.dma_start(dyl_sb[:], bslice("dyl", rows=96).bitcast(BF16))
        nc.sync.dma_start(encl_sb[:], bslice("encl").bitcast(BF16))
        trig_sb = persist.tile([128, 4 * (2 * (T // 64) + 128)], FP32)
        nc.sync.dma_start(trig_sb[:], bslice("trig"))
        nc.sync.dma_start(triu_sb[:], bslice("triu").bitcast(BF16))
        nc.sync.dma_start(id_sb[:], bslice("ident").bitcast(BF16))
        nc.sync.dma_start(rwt_sb[:], bslice("rwt").bitcast(BF16))
        nc.sync.dma_start(sel_sb[:], bslice("sel").bitcast(BF16))
        nc.gpsimd.memset(mbias_sb[:], -M_BIAS)
        nc.gpsimd.memset(eps_sb[:], EPS)
        nc.gpsimd.memset(onesA[:], 1.0 / 128.0)
        nc.gpsimd.memset(onesB[:], 1.0)
        vnh3 = vnh[:].rearrange("p (g c) -> p g c", c=96)
        nc.gpsimd.memset(vnh3[:, :, 0:1], 1.0)
        if apply_g1b1:
            g1c_sb = persist.tile([128, 1], FP32); nc.sync.dma_start(g1c_sb[:], bslice("g1c"))
            b1c_sb = persist.tile([128, 1], FP32); nc.sync.dma_start(b1c_sb[:], bslice("b1c"))
        if apply_g2b2:
            g2f_sb = persist.tile([128, 1], FP32); nc.sync.dma_start(g2f_sb[:], bslice("g2f"))
            b2f_sb = persist.tile([128, 1], FP32); nc.sync.dma_start(b2f_sb[:], bslice("b2f"))

        # Build rope sin/cos tables on device via sin(A+B)/cos(A+B) identities.
        THI = T // 64
        CPW = 2 * THI + 128
        t1v = dv_sb[:].rearrange("p (a b) -> p a b", b=64)
        t2v = dv2_sb[:].rearrange("p (a b) -> p a b", b=64)
        for cp in range(4):
            o = cp * CPW
            sA = trig_sb[:, o:o + THI, None].to_broadcast((128, THI, 64))
            cA = trig_sb[:, o + THI:o + 2 * THI, None].to_broadcast((128, THI, 64))
            sB = trig_sb[:, None, o + 2 * THI:o + 2 * THI + 64].to_broadcast((128, THI, 64))
            cB = trig_sb[:, None, o + 2 * THI + 64:o + CPW].to_broadcast((128, THI, 64))
            sin_o = sin_sb[:, cp * T:(cp + 1) * T].rearrange("p (a b) -> p a b", b=64)
            cos_o = cos_sb[:, cp * T:(cp + 1) * T].rearrange("p (a b) -> p a b", b=64)
            nc.vector.tensor_tensor(t1v, sA, cB, AL.mult)
            nc.vector.tensor_tensor(t2v, cA, sB, AL.mult)
            nc.vector.tensor_tensor(sin_o, t1v, t2v, AL.add)
            nc.vector.tensor_tensor(t1v, cA, cB, AL.mult)
            nc.vector.tensor_tensor(t2v, sA, sB, AL.mult)
            nc.vector.tensor_tensor(cos_o, t1v, t2v, AL.subtract)

        def ln_pass(m):
            """Per-macro LN stats on vT; returns (rs_b, mr_b) broadcast tiles."""
            tm = slice(m * 512, (m + 1) * 512)
            nc.vector.tensor_copy(vb_sb[:, tm], vT[:, tm])
            nc.scalar.activation(sq_sb[:, tm], vT[:, tm], AF.Square, scale=RSQRT_D)
            mu_ps = ps_w.tile([1, 512], FP32, tag="w", name="mu")
            nc.tensor.matmul(mu_ps[:], onesA[:], vb_sb[:, tm], start=True, stop=True)
            m2_ps = ps_w.tile([1, 512], FP32, tag="w", name="m2")
            nc.tensor.matmul(m2_ps[:], onesB[:], sq_sb[:, tm], start=True, stop=True)
            msq = stat.tile([1, 512], FP32, tag="st1")
            nc.scalar.activation(msq[:], mu_ps[:], AF.Square)
            var = stat.tile([1, 512], FP32, tag="st1")
            nc.vector.tensor_tensor(var[:], m2_ps[:], msq[:], AL.subtract)
            lnv = stat.tile([1, 512], FP32, tag="st1")
            nc.scalar.activation(lnv[:], var[:], AF.Ln, bias=eps_sb[0:1, :], scale=1.0)
            rs = stat.tile([1, 512], FP32, tag="st2")
            nc.scalar.activation(rs[:], lnv[:], AF.Exp, scale=-0.5)
            mr = stat.tile([1, 512], FP32, tag="st2")
            nc.vector.tensor_tensor(mr[:], mu_ps[:], rs[:], AL.mult)
            rs_b = stat.tile([128, 512], FP32, tag="stb")
            nc.gpsimd.partition_broadcast(rs_b[:], rs[:], channels=128)
            mr_b = stat.tile([128, 512], FP32, tag="stb")
            nc.gpsimd.partition_broadcast(mr_b[:], mr[:], channels=128)
            return rs_b, mr_b

        for l in range(n_layers):
            # ---------------- ln1 -> vnTc (compact, bf16) + vnh ----------------
            for m in range(NM):
                tm = slice(m * 512, (m + 1) * 512)
                rs_b, mr_b = ln_pass(m)
                selv = ps_w.tile([128, 512], FP32, tag="w", name="selv")
                nc.tensor.matmul(selv[:], sel_sb[:], vb_sb[:, tm], start=True, stop=True)
                tmp = work.tile([128, 512], FP32, tag="lnt")
                nc.vector.tensor_tensor(tmp[:], selv[:], rs_b[:], AL.mult)
                nc.vector.tensor_tensor(vnTc[:, tm], tmp[:], mr_b[:], AL.subtract)
                if apply_g1b1:
                    nc.vector.tensor_scalar_mul(vnTc[:, tm], vnTc[:, tm], g1c_sb[:])
                    nc.vector.tensor_scalar(vnTc[:, tm], vnTc[:, tm], b1c_sb[:], AL.add)
                for q in range(4):
                    tt = 4 * m + q
                    ptr = ps_t.tile([128, 64], BF16, tag="tr")
                    nc.tensor.transpose(ptr[:], vnTc[0:64, tt * 128:(tt + 1) * 128],
                                        id_sb[0:64, 0:64])
                    nc.vector.tensor_copy(vnh3[:, tt, 32:96], ptr[:])

            # ------- per-head: x -> rope -> attention -> yi -> dv -------
            for hl in range(2):
                # X = relu(vn @ dx), 2-way row-packed (chunk pairs c, c+4)
                for cp in range(4):
                    for m in range(NM):
                        tm = slice(m * 512, (m + 1) * 512)
                        for (rg, cc) in ((32 * hl, cp), (64 + 32 * hl, cp + 4)):
                            px = ps_w.tile([128, 512], FP32, tag="w", name="px")
                            nc.tensor.matmul(px[:],
                                             dxl2_sb[rg:rg + 32, cc * 128:(cc + 1) * 128],
                                             vnTc[rg:rg + 32, tm], start=True, stop=True,
                                             tile_position=(rg, 0))
                            nc.scalar.activation(
                                x_sb[:, cc * T + m * 512: cc * T + (m + 1) * 512],
                                px[:], AF.Relu)
                    # rope for this chunk pair
                    xe = x_sb[:, cp * T:(cp + 1) * T]
                    xo = x_sb[:, (cp + 4) * T:(cp + 5) * T]
                    co = cos_sb[:, cp * T:(cp + 1) * T]
                    si = sin_sb[:, cp * T:(cp + 1) * T]
                    t1 = ropet.tile([128, T], BF16, tag="r1")
                    t2 = ropet.tile([128, T], BF16, tag="r2")
                    nc.vector.tensor_tensor(t1[:], xe, co, AL.mult)
                    nc.vector.tensor_tensor(t2[:], xo, si, AL.mult)
                    nc.vector.tensor_tensor(xr_sb[:, cp * T:(cp + 1) * T], t1[:], t2[:],
                                            AL.subtract)
                    t3 = ropet.tile([128, T], BF16, tag="r1")
                    t4 = ropet.tile([128, T], BF16, tag="r2")
                    nc.vector.tensor_tensor(t3[:], xe, si, AL.mult)
                    nc.vector.tensor_tensor(t4[:], xo, co, AL.mult)
                    nc.vector.tensor_tensor(xr_sb[:, (cp + 4) * T:(cp + 5) * T], t3[:], t4[:],
                                            AL.add)

                # attention (transposed scores) for this head
                for m in range(NM):
                    aT_ps = ps_a.tile([96, 512], FP32, tag="a")
                    nblk = 4 * m + 4
                    for k in range(nblk):
                        j = k - 4 * m
                        if j < 0:
                            toff, w = m * 512, 512
                        else:
                            toff, w = m * 512 + 128 * j, 512 - 128 * j
                        pss = ps_s.tile([128, 512], FP32, tag="s")
                        for c in range(NCH):
                            nc.tensor.matmul(
                                pss[:, :w],
                                xr_sb[:, c * T + 128 * k: c * T + 128 * (k + 1)],
                                xr_sb[:, c * T + toff: c * T + toff + w],
                                start=(c == 0), stop=(c == NCH - 1))
                        ex = work.tile([128, 512], BF16, tag="ex")
                        nc.scalar.activation(ex[:, :w], pss[:, :w], AF.Exp,
                                             bias=mbias_sb[:], scale=1.0)
                        if j >= 0:
                            nc.vector.tensor_tensor(ex[:, 0:128], ex[:, 0:128],
                                                    triu_sb[:], AL.mult)
                        nc.tensor.matmul(aT_ps[:, toff - m * 512: toff - m * 512 + w],
                                         vnh3[:, k, 0:96], ex[:, :w],
                                         start=(k == 0), stop=(k == nblk - 1),
                                         skip_group_check=True)
                    # normalize this head's rows by 1/rowsum = exp(-ln(rowsum))
                    r1 = stat.tile([1, 512], FP32, tag="rq")
                    nc.scalar.activation(r1[:], aT_ps[0:1, :], AF.Ln)
                    r2 = stat.tile([1, 512], FP32, tag="rq")
                    nc.scalar.activation(r2[:], r1[:], AF.Exp, scale=-1.0)
                    rinv_b = stat.tile([96, 512], FP32, tag="stb")
                    nc.gpsimd.partition_broadcast(rinv_b[:], r2[:], channels=96)
                    rr = slice(32 + 32 * hl, 64 + 32 * hl)
                    nc.vector.tensor_tensor(
                        aT_sb[rr, m * 512:(m + 1) * 512],
                        aT_ps[rr, :], rinv_b[rr, :], AL.mult)

                # YI = relu(a @ dy) * x (into xr_sb); dv^T accumulate
                for m in range(NM):
                    tm = slice(m * 512, (m + 1) * 512)
                    dvp = ps_w.tile([128, 512], FP32, tag="w", name="dvp")
                    for c in range(NCH):
                        py = ps_w.tile([128, 512], FP32, tag="w", name="py")
                        nc.tensor.matmul(py[:],
                                         dyl_sb[32 + 32 * hl:64 + 32 * hl, c * 128:(c + 1) * 128],
                                         aT_sb[32 + 32 * hl:64 + 32 * hl, tm],
                                         start=True, stop=True)
                        rl = work.tile([128, 512], BF16, tag="rl")
                        nc.scalar.activation(rl[:], py[:], AF.Relu)
                        nc.vector.tensor_tensor(
                            xr_sb[:, c * T + m * 512: c * T + (m + 1) * 512], rl[:],
                            x_sb[:, c * T + m * 512: c * T + (m + 1) * 512], AL.mult)
                        nc.tensor.matmul(
                            dvp[:],
                            encl_sb[:, (hl * NCH + c) * D:(hl * NCH + c + 1) * D],
                            xr_sb[:, c * T + m * 512: c * T + (m + 1) * 512],
                            start=(c == 0), stop=(c == NCH - 1),
                            skip_group_check=True)
                    if hl == 0:
                        nc.vector.tensor_copy(dv_sb[:, tm], dvp[:])
                    else:
                        nc.vector.tensor_tensor(dv_sb[:, tm], dv_sb[:, tm], dvp[:], AL.add)

            # ---------------- pair all-reduce of dv; v += dv_tot -------------
            inb = dramp.tile([128, T], FP32, tag="inb")
            outb = dramp.tile([128, T], FP32, tag="outb")
            nc.gpsimd.dma_start(inb[:], dv_sb[:])
            if use_collective:
                nc.gpsimd.collective_compute(
                    "AllReduce", AL.add,
                    replica_groups=[[0, 1], [2, 3], [4, 5], [6, 7]],
                    ins=[inb[:].opt()], outs=[outb[:].opt()])
            rb = outb if use_collective else inb
            nc.gpsimd.dma_start(dv2_sb[:], rb[:])

            # ---------------- v += dv; ln2: v = v + ln(v) ----------------
            for m in range(NM):
                tm = slice(m * 512, (m + 1) * 512)
                nc.vector.tensor_tensor(vT[:, tm], vT[:, tm], dv2_sb[:, tm], AL.add)
                rs_b, mr_b = ln_pass(m)
                t0 = work.tile([128, 512], FP32, tag="lnt")
                nc.vector.tensor_tensor(t0[:], vT[:, tm], rs_b[:], AL.mult)
                t1 = work.tile([128, 512], FP32, tag="lnt2")
                nc.vector.tensor_tensor(t1[:], t0[:], mr_b[:], AL.subtract)
                if apply_g2b2:
                    nc.vector.tensor_scalar_mul(t1[:], t1[:], g2f_sb[:])
                    nc.vector.tensor_scalar(t1[:], t1[:], b2f_sb[:], AL.add)
                nc.vector.tensor_tensor(vT[:, tm], vT[:, tm], t1[:], AL.add)

        # ---------------- logits = v @ readout^T ----------------
        for m in range(NM):
            tm = slice(m * 512, (m + 1) * 512)
            nc.vector.tensor_copy(vb_sb[:, tm], vT[:, tm])
        for tt in range(NT):
            pl = ps_w.tile([128, VOCAB], FP32, tag="w", name="pl")
            nc.tensor.matmul(pl[:], vb_sb[:, tt * 128:(tt + 1) * 128], rwt_sb[:],
                             start=True, stop=True)
            lf = work.tile([128, VOCAB], FP32, tag="lf")
            nc.vector.tensor_copy(lf[:], pl[:])
            nc.sync.dma_start(logits_e[tt * 128:(tt + 1) * 128, :], lf[:])

    nc.compile()
    return nc


# ---------------------------------------------------------------------------
# host-side preparation
# ---------------------------------------------------------------------------

def _prep_core_inputs(inputs, core, T):
    b = min(core // 2, np.asarray(inputs["idx"]).shape[0] - 1)
    heads = [0, 1] if core % 2 == 0 else [2, 3]

    idx = np.asarray(inputs["idx"])
    wte = np.asarray(inputs["wte"], np.float32)
    encoder = np.asarray(inputs["encoder"], np.float32)
    decoder_x = np.asarray(inputs["decoder_x"], np.float32)
    decoder_y = np.asarray(inputs["decoder_y"], np.float32)
    readout_w = np.asarray(inputs["readout_w"], np.float32)

    perm = np.concatenate([np.arange(0, NH, 2), np.arange(1, NH, 2)])

    v0T = wte[idx[b, :T]].astype(np.float32).T                 # [D, T]

    dxh = [decoder_x[h][:, perm].astype(BF) for h in heads]    # [32,1024] each
    dyh = [decoder_y[h][:, perm].astype(BF) for h in heads]
    dxl2 = np.concatenate([dxh[0], dxh[1], dxh[0], dxh[1]], 0)  # [128,1024]
    dyl = np.concatenate([np.zeros((32, NH), BF), dyh[0], dyh[1]], 0)  # [96,1024]

    encl = np.zeros((128, 2 * NCH * D), BF)
    encr = encoder.reshape(H, NH, D)
    for hl, h in enumerate(heads):
        ehp = encr[h][perm, :]                                  # [NH, D]
        for c in range(NCH):
            encl[:, (hl * NCH + c) * D:(hl * NCH + c + 1) * D] = \
                ehp[c * 128:(c + 1) * 128, :].astype(BF)

    div = np.exp(np.arange(0, NH, 2, dtype=np.float64) * (-np.log(10000.0) / NH))
    THI = T // 64
    CPW = 2 * THI + 128
    trig = np.zeros((128, 4 * CPW), np.float32)
    thi = np.arange(THI, dtype=np.float64) * 64.0
    tlo = np.arange(64, dtype=np.float64)
    for cp in range(4):
        dk = div[cp * 128:(cp + 1) * 128][:, None]              # [128,1]
        o = cp * CPW
        trig[:, o:o + THI] = np.sin(dk * thi)
        trig[:, o + THI:o + 2 * THI] = np.cos(dk * thi)
        trig[:, o + 2 * THI:o + 2 * THI + 64] = np.sin(dk * tlo)
        trig[:, o + 2 * THI + 64:o + CPW] = np.cos(dk * tlo)

    triu = np.triu(np.ones((128, 128), np.float32), 0).astype(BF)  # keep t>=s
    ident = np.eye(128, dtype=np.float32).astype(BF)
    rwt = readout_w.T.astype(BF)                                # [128, 256]
    sel = np.zeros((128, 128), np.float32)
    for j in range(128):
        hl = (j // 32) % 2
        sel[heads[hl] * DH + (j % 32), j] = 1.0
    sel = sel.astype(BF)

    g1 = np.asarray(inputs["ln1_g"], np.float32); b1 = np.asarray(inputs["ln1_b"], np.float32)
    g2 = np.asarray(inputs["ln2_g"], np.float32); b2 = np.asarray(inputs["ln2_b"], np.float32)
    a1 = not (np.all(g1 == 1.0) and np.all(b1 == 0.0))
    a2 = not (np.all(g2 == 1.0) and np.all(b2 == 0.0))

    offs = _blob_offsets(T, a1, a2)
    blob = np.zeros((128, offs["_total"]), np.float32)

    def put32(name, arr, rows=slice(0, 128)):
        o, w = offs[name]
        blob[rows, o:o + w] = arr
    def putbf(name, arr_bf, rows=slice(0, 128)):
        o, w = offs[name]
        blob[rows, o:o + arr_bf.shape[1] // 2] = \
            np.ascontiguousarray(arr_bf).view(np.float32)

    putbf("v0T", v0T.astype(BF))
    putbf("dxl2", dxl2)
    putbf("dyl", dyl, rows=slice(0, 96))
    putbf("encl", encl)
    put32("trig", trig)
    putbf("triu", triu)
    putbf("ident", ident)
    putbf("rwt", rwt)
    putbf("sel", sel)
    if a1:
        g1c = np.array([g1[heads[(j // 32) % 2] * DH + (j % 32)] for j in range(128)])
        b1c = np.array([b1[heads[(j // 32) % 2] * DH + (j % 32)] for j in range(128)])
        put32("g1c", g1c[:, None])
        put32("b1c", b1c[:, None])
    if a2:
        put32("g2f", g2[:, None])
        put32("b2f", b2[:, None])
    return {"blob": blob}


_BUILT = {}


def _get_kernel(T, apply_g1b1, apply_g2b2):
    key = (T, apply_g1b1, apply_g2b2)
    if key not in _BUILT:
        _BUILT[key] = build_kernel(T, apply_g1b1, apply_g2b2)
    return _BUILT[key]


def kernel(**inputs) -> np.ndarray:
    idx = np.asarray(inputs["idx"])
    B, T = idx.shape
    g1 = np.asarray(inputs["ln1_g"], np.float32); b1 = np.asarray(inputs["ln1_b"], np.float32)
    g2 = np.asarray(inputs["ln2_g"], np.float32); b2 = np.asarray(inputs["ln2_b"], np.float32)
    a1 = not (np.all(g1 == 1.0) and np.all(b1 == 0.0))
    a2 = not (np.all(g2 == 1.0) and np.all(b2 == 0.0))

    nc = _get_kernel(T, a1, a2)
    in_maps = [_prep_core_inputs(inputs, c, T) for c in range(NCORES)]
    res = run_bass_kernel_spmd(nc, in_maps, list(range(NCORES)))
    out = np.stack([res.results[2 * b]["logits"] for b in range(B)], 0)
    return out.astype(np.float32)


# revision 12
# speedup vs baseline: 1.4941x; 1.4941x over previous
"""Trainium2 Bass kernel for the 4-layer dense transformer (nn_BDH_GPU_65326452572468).

Sharding: 8 cores = 4 batches x 2 head-pairs. Core c handles batch c//2 and
heads {0,1} (c even) or {2,3} (c odd). Per layer, each core computes its two
heads' attention and dv contribution; dv is all-reduced within the core pair,
after which v stays replicated. Logits are taken from the even core of each pair.

v2 design notes (vs the v1 baseline):
- v lives TRANSPOSED on device: vT [d=128 partitions, T free]. LN stats are
  computed with ones-weight matmuls over the partition (d) axis; the per-token
  affine is applied via gpsimd partition_broadcast + two DVE ops.
- Attention computes scores TRANSPOSED (S^T[s,t] blocks) by swapping matmul
  operands, so P^T is produced directly and the ~1100 PE transposes of the v1
  kernel disappear. Softmax row-sums come free from a ones-column appended to
  the V operand (vnh_aug [s, 65]); normalization scales a^T per-column after
  accumulation (relu commutes with the positive scale).
- Causal masking: diagonal s-blocks stream only their valid t-range
  (N in {512,384,256,128}) and the one triangular 128-wide subtile is masked
  by multiplying exp by a 0/1 triu mask (bf16, cheap).
- x = relu(vn@dx) / yi-gemms use tile_position row-packing (K=32 -> 4-way /
  2-way concurrent in the 128x128 PE array).
- All matmuls bf16 with fp32 PSUM; softmax uses the constant-bias trick
  (scores bounded ~12.2, bias 16).
"""
import sys
import numpy as np

sys.path.insert(0, "/opt/trn_rl_repo")

import ml_dtypes

import concourse.bass as bass
import concourse.mybir as mybir
import concourse.tile as tile
from concourse import bacc
from concourse.bass_utils import run_bass_kernel_spmd

BF = ml_dtypes.bfloat16
FP32 = mybir.dt.float32
BF16 = mybir.dt.bfloat16
AL = mybir.AluOpType
AF = mybir.ActivationFunctionType
AX = mybir.AxisListType

D = 128
H = 4
L = 4
N = 4096
VOCAB = 256
DH = 32          # D // H
NH = 1024        # N // H
EPS = 1e-5
M_BIAS = 16.0    # constant softmax shift (max observed score ~12.2)
NCORES = 8
NCH = NH // 128  # 8 i-chunks per head
RSQRT_D = float(1.0 / np.sqrt(128.0))


def _blob_offsets(T, apply_g1b1, apply_g2b2):
    """Word offsets (per 128-partition row) of each packed constant."""
    offs, o = {}, 0
    def add(name, words):
        nonlocal o
        offs[name] = (o, words)
        o += words
    add("v0T", T // 2)               # bf16 [128, T]  (d-major v0)
    add("dxl2", NH // 2)             # bf16 [128, NH]: h0,h1,h0,h1 in 32-row groups
    add("dyl", NH // 2)              # bf16 rows 32-63 h0, 64-95 h1
    add("encl", NCH * D)             # bf16 [128, 2*NCH*D]
    add("trig", 4 * (2 * (T // 64) + 128))  # fp32 rope seed tables
    add("triu", 64)                  # bf16 [128,128] 0/1, keep t>=s
    add("ident", 64)                 # bf16 [128,128]
    add("rwt", VOCAB // 2)           # bf16 [128, VOCAB]
    add("sel", 64)                   # bf16 [128,128] d -> compact-row selection
    if apply_g1b1:
        add("g1c", 1); add("b1c", 1)
    if apply_g2b2:
        add("g2f", 1); add("b2f", 1)
    offs["_total"] = o
    return offs


def _prefer_combined_act_table():
    """Reorder the ACT table-set dict so natural_log_exp_and_others (which
    contains every function this kernel uses: exp, ln, relu, square, copy)
    is picked for all activations -- avoids per-macro table thrash."""
    import concourse.hw_specs as hw_specs
    import concourse.bacc as bacc_mod
    if getattr(hw_specs, "_combined_act_patch", False):
        return
    orig = hw_specs.get_activation_tables

    def patched(arch):
        # Preserve set order (act_func_set_id is an index into the original
        # act_info.json list); steer selection by removing this kernel's
        # functions from every other set.
        tabs = orig(arch)
        pref = "natural_log_exp_and_others"
        if pref not in tabs:
            return tabs
        mine = tabs[pref]
        return {k: (v if k == pref else (v - mine)) for k, v in tabs.items()}

    hw_specs.get_activation_tables = patched
    bacc_mod.get_activation_tables = patched
    hw_specs._combined_act_patch = True


def build_kernel(T: int, apply_g1b1: bool, apply_g2b2: bool,
                 use_collective: bool = True, n_layers: int = L):
    _prefer_combined_act_table()
    NT = T // 128
    NM = T // 512
    nc = bacc.Bacc("TRN2", target_bir_lowering=False, debug=False,
                   num_devices=NCORES)

    offs = _blob_offsets(T, apply_g1b1, apply_g2b2)
    blob_e = nc.dram_tensor("blob", [128, offs["_total"]], FP32,
                            kind="ExternalInput").ap()
    logits_e = nc.dram_tensor("logits", [T, VOCAB], FP32, kind="ExternalOutput").ap()

    def bslice(name, rows=128):
        o, w = offs[name]
        return blob_e[0:rows, o:o + w]

    from contextlib import ExitStack
    with tile.TileContext(nc) as tc, ExitStack() as stack:
        persist = stack.enter_context(tc.tile_pool(name="persist", bufs=1))
        work = stack.enter_context(tc.tile_pool(name="work", bufs=3))
        ropet = stack.enter_context(tc.tile_pool(name="ropet", bufs=2))
        stat = stack.enter_context(tc.tile_pool(name="stat", bufs=2))
        dramp = stack.enter_context(tc.tile_pool(name="dramp", bufs=2, space="DRAM"))
        ps_s = stack.enter_context(tc.tile_pool(name="ps_s", bufs=2, space="PSUM"))
        ps_a = stack.enter_context(tc.tile_pool(name="ps_a", bufs=2, space="PSUM"))
        ps_w = stack.enter_context(tc.tile_pool(name="ps_w", bufs=3, space="PSUM"))
        ps_t = stack.enter_context(tc.tile_pool(name="ps_t", bufs=1, space="PSUM"))

        # ---- persistent SBUF state ----
        vT = persist.tile([128, T], FP32)             # v transposed [d, t]
        dv_sb = persist.tile([128, T], FP32)          # dv^T (also rope scratch)
        dv2_sb = persist.tile([128, T], FP32)         # reduced dv^T (also rope scratch)
        vb_sb = persist.tile([128, T], BF16)          # bf16 cast of vT
        sq_sb = persist.tile([128, T], BF16)          # (vT^2)/128 bf16
        vnTc = persist.tile([128, T], BF16)           # compact ln1(v): h0,h1,h0,h1
        vnh = persist.tile([128, NT * 96], BF16)      # [s,96]: col0=ones, 32-95=vn
        aT_sb = persist.tile([96, T], BF16)           # a^T rows 32-63 h0, 64-95 h1
        x_sb = persist.tile([128, NCH * T], BF16)     # x (one head), chunk c at c*T
        xr_sb = persist.tile([128, NCH * T], BF16)    # rope(x); reused as yi
        cos_sb = persist.tile([128, 4 * T], BF16)
        sin_sb = persist.tile([128, 4 * T], BF16)
        dxl2_sb = persist.tile([128, NH], BF16)
        dyl_sb = persist.tile([96, NH], BF16)         # rows 32-63 h0, 64-95 h1
        encl_sb = persist.tile([128, 2 * NCH * D], BF16)
        sel_sb = persist.tile([128, 128], BF16)
        triu_sb = persist.tile([128, 128], BF16)
        id_sb = persist.tile([128, 128], BF16)
        rwt_sb = persist.tile([D, VOCAB], BF16)
        mbias_sb = persist.tile([128, 1], FP32)
        eps_sb = persist.tile([128, 1], FP32)
        onesA = persist.tile([128, 1], BF16)          # 1/128
        onesB = persist.tile([128, 1], BF16)          # 1.0

        nc.sync.dma_start(vb_sb[:], bslice("v0T").bitcast(BF16))
        nc.vector.tensor_copy(vT[:], vb_sb[:])
        nc.sync.dma_start(dxl2_sb[:], bslice("dxl2").bitcast(BF16))
        nc.sync.dma_start(dyl_sb[:], bslice("dyl", rows=96).bitcast(BF16))
        nc.sync.dma_start(encl_sb[:], bslice("encl").bitcast(BF16))
        trig_sb = persist.tile([128, 4 * (2 * (T // 64) + 128)], FP32)
        nc.sync.dma_start(trig_sb[:], bslice("trig"))
        nc.sync.dma_start(triu_sb[:], bslice("triu").bitcast(BF16))
        nc.sync.dma_start(id_sb[:], bslice("ident").bitcast(BF16))
        nc.sync.dma_start(rwt_sb[:], bslice("rwt").bitcast(BF16))
        nc.sync.dma_start(sel_sb[:], bslice("sel").bitcast(BF16))
        nc.gpsimd.memset(mbias_sb[:], -M_BIAS)
        nc.gpsimd.memset(eps_sb[:], EPS)
        nc.gpsimd.memset(onesA[:], 1.0 / 128.0)
        nc.gpsimd.memset(onesB[:], 1.0)
        vnh3 = vnh[:].rearrange("p (g c) -> p g c", c=96)
        nc.gpsimd.memset(vnh3[:, :, 0:1], 1.0)
        if apply_g1b1:
            g1c_sb = persist.tile([128, 1], FP32); nc.sync.dma_start(g1c_sb[:], bslice("g1c"))
            b1c_sb = persist.tile([128, 1], FP32); nc.sync.dma_start(b1c_sb[:], bslice("b1c"))
        if apply_g2b2:
            g2f_sb = persist.tile([128, 1], FP32); nc.sync.dma_start(g2f_sb[:], bslice("g2f"))
            b2f_sb = persist.tile([128, 1], FP32); nc.sync.dma_start(b2f_sb[:], bslice("b2f"))

        # Build rope sin/cos tables on device via sin(A+B)/cos(A+B) identities.
        THI = T // 64
        CPW = 2 * THI + 128
        t1v = dv_sb[:].rearrange("p (a b) -> p a b", b=64)
        t2v = dv2_sb[:].rearrange("p (a b) -> p a b", b=64)
        for cp in range(4):
            o = cp * CPW
            sA = trig_sb[:, o:o + THI, None].to_broadcast((128, THI, 64))
            cA = trig_sb[:, o + THI:o + 2 * THI, None].to_broadcast((128, THI, 64))
            sB = trig_sb[:, None, o + 2 * THI:o + 2 * THI + 64].to_broadcast((128, THI, 64))
            cB = trig_sb[:, None, o + 2 * THI + 64:o + CPW].to_broadcast((128, THI, 64))
            sin_o = sin_sb[:, cp * T:(cp + 1) * T].rearrange("p (a b) -> p a b", b=64)
            cos_o = cos_sb[:, cp * T:(cp + 1) * T].rearrange("p (a b) -> p a b", b=64)
            nc.vector.tensor_tensor(t1v, sA, cB, AL.mult)
            nc.vector.tensor_tensor(t2v, cA, sB, AL.mult)
            nc.vector.tensor_tensor(sin_o, t1v, t2v, AL.add)
            nc.vector.tensor_tensor(t1v, cA, cB, AL.mult)
            nc.vector.tensor_tensor(t2v, sA, sB, AL.mult)
            nc.vector.tensor_tensor(cos_o, t1v, t2v, AL.subtract)

        def ln_pass(m):
            """Per-macro LN stats on vT; returns (rs_b, mr_b) broadcast tiles."""
            tm = slice(m * 512, (m + 1) * 512)
            nc.vector.tensor_copy(vb_sb[:, tm], vT[:, tm])
            nc.scalar.activation(sq_sb[:, tm], vT[:, tm], AF.Square, scale=RSQRT_D)
            mu_ps = ps_w.tile([1, 512], FP32, tag="w", name="mu")
            nc.tensor.matmul(mu_ps[:], onesA[:], vb_sb[:, tm], start=True, stop=True)
            m2_ps = ps_w.tile([1, 512], FP32, tag="w", name="m2")
            nc.tensor.matmul(m2_ps[:], onesB[:], sq_sb[:, tm], start=True, stop=True)
            msq = stat.tile([1, 512], FP32, tag="st1")
            nc.scalar.activation(msq[:], mu_ps[:], AF.Square)
            var = stat.tile([1, 512], FP32, tag="st1")
            nc.vector.tensor_tensor(var[:], m2_ps[:], msq[:], AL.subtract)
            lnv = stat.tile([1, 512], FP32, tag="st1")
            nc.scalar.activation(lnv[:], var[:], AF.Ln, bias=eps_sb[0:1, :], scale=1.0)
            rs = stat.tile([1, 512], FP32, tag="st2")
            nc.scalar.activation(rs[:], lnv[:], AF.Exp, scale=-0.5)
            mr = stat.tile([1, 512], FP32, tag="st2")
            nc.vector.tensor_tensor(mr[:], mu_ps[:], rs[:], AL.mult)
            rs_b = stat.tile([128, 512], FP32, tag="stb")
            nc.gpsimd.partition_broadcast(rs_b[:], rs[:], channels=128)
            mr_b = stat.tile([128, 512], FP32, tag="stb")
            nc.gpsimd.partition_broadcast(mr_b[:], mr[:], channels=128)
            return rs_b, mr_b

        for l in range(n_layers):
            # ---------------- ln1 -> vnTc (compact, bf16) + vnh ----------------
            for m in range(NM):
                tm = slice(m * 512, (m + 1) * 512)
                rs_b, mr_b = ln_pass(m)
                selv = ps_w.tile([128, 512], FP32, tag="w", name="selv")
                nc.tensor.matmul(selv[:], sel_sb[:], vb_sb[:, tm], start=True, stop=True)
                tmp = work.tile([128, 512], FP32, tag="lnt")
                nc.vector.tensor_tensor(tmp[:], selv[:], rs_b[:], AL.mult)
                nc.vector.tensor_tensor(vnTc[:, tm], tmp[:], mr_b[:], AL.subtract)
                if apply_g1b1:
                    nc.vector.tensor_scalar_mul(vnTc[:, tm], vnTc[:, tm], g1c_sb[:])
                    nc.vector.tensor_scalar(vnTc[:, tm], vnTc[:, tm], b1c_sb[:], AL.add)
                for q in range(4):
                    tt = 4 * m + q
                    ptr = ps_t.tile([128, 64], BF16, tag="tr")
                    nc.tensor.transpose(ptr[:], vnTc[0:64, tt * 128:(tt + 1) * 128],
                                        id_sb[0:64, 0:64])
                    nc.vector.tensor_copy(vnh3[:, tt, 32:96], ptr[:])

            # ------- per-head: x -> rope -> attention -> yi -> dv -------
            for hl in range(2):
                # X = relu(vn @ dx), 2-way row-packed (chunk pairs c, c+4)
                for cp in range(4):
                    for m in range(NM):
                        tm = slice(m * 512, (m + 1) * 512)
                        for (rg, cc) in ((32 * hl, cp), (64 + 32 * hl, cp + 4)):
                            px = ps_w.tile([128, 512], FP32, tag="w", name="px")
                            nc.tensor.matmul(px[:],
                                             dxl2_sb[rg:rg + 32, cc * 128:(cc + 1) * 128],
                                             vnTc[rg:rg + 32, tm], start=True, stop=True,
                                             tile_position=(rg, 0))
                            nc.scalar.activation(
                                x_sb[:, cc * T + m * 512: cc * T + (m + 1) * 512],
                                px[:], AF.Relu)
                    # rope for this chunk pair
                    xe = x_sb[:, cp * T:(cp + 1) * T]
                    xo = x_sb[:, (cp + 4) * T:(cp + 5) * T]
                    co = cos_sb[:, cp * T:(cp + 1) * T]
                    si = sin_sb[:, cp * T:(cp + 1) * T]
                    t1 = ropet.tile([128, T], BF16, tag="r1")
                    t2 = ropet.tile([128, T], BF16, tag="r2")
                    nc.vector.tensor_tensor(t1[:], xe, co, AL.mult)
                    nc.vector.tensor_tensor(t2[:], xo, si, AL.mult)
                    nc.vector.tensor_tensor(xr_sb[:, cp * T:(cp + 1) * T], t1[:], t2[:],
                                            AL.subtract)
                    t3 = ropet.tile([128, T], BF16, tag="r1")
                    t4 = ropet.tile([128, T], BF16, tag="r2")
                    nc.vector.tensor_tensor(t3[:], xe, si, AL.mult)
                    nc.vector.tensor_tensor(t4[:], xo, co, AL.mult)
                    nc.vector.tensor_tensor(xr_sb[:, (cp + 4) * T:(cp + 5) * T], t3[:], t4[:],
                                            AL.add)

                # attention (transposed scores) for this head
                for m in range(NM):
                    aT_ps = ps_a.tile([96, 512], FP32, tag="a")
                    nblk = 4 * m + 4
                    for k in range(nblk):
                        j = k - 4 * m
                        if j < 0:
                            toff, w = m * 512, 512
                        else:
                            toff, w = m * 512 + 128 * j, 512 - 128 * j
                        pss = ps_s.tile([128, 512], FP32, tag="s")
                        for c in range(NCH):
                            nc.tensor.matmul(
                                pss[:, :w],
                                xr_sb[:, c * T + 128 * k: c * T + 128 * (k + 1)],
                                xr_sb[:, c * T + toff: c * T + toff + w],
                                start=(c == 0), stop=(c == NCH - 1))
                        ex = work.tile([128, 512], BF16, tag="ex")
                        nc.scalar.activation(ex[:, :w], pss[:, :w], AF.Exp,
                                             bias=mbias_sb[:], scale=1.0)
                        if j >= 0:
                            nc.vector.tensor_tensor(ex[:, 0:128], ex[:, 0:128],
                                                    triu_sb[:], AL.mult)
                        nc.tensor.matmul(aT_ps[:, toff - m * 512: toff - m * 512 + w],
                                         vnh3[:, k, 0:96], ex[:, :w],
                                         start=(k == 0), stop=(k == nblk - 1),
                                         skip_group_check=True)
                    # normalize this head's rows by 1/rowsum = exp(-ln(rowsum))
                    r1 = stat.tile([1, 512], FP32, tag="rq")
                    nc.scalar.activation(r1[:], aT_ps[0:1, :], AF.Ln)
                    r2 = stat.tile([1, 512], FP32, tag="rq")
                    nc.scalar.activation(r2[:], r1[:], AF.Exp, scale=-1.0)
                    rinv_b = stat.tile([96, 512], FP32, tag="stb")
                    nc.gpsimd.partition_broadcast(rinv_b[:], r2[:], channels=96)
                    rr = slice(32 + 32 * hl, 64 + 32 * hl)
                    nc.vector.tensor_tensor(
                        aT_sb[rr, m * 512:(m + 1) * 512],
                        aT_ps[rr, :], rinv_b[rr, :], AL.mult)

                # YI = relu(a @ dy) * x (into xr_sb); dv^T accumulate
                for m in range(NM):
                    tm = slice(m * 512, (m + 1) * 512)
                    dvp = ps_w.tile([128, 512], FP32, tag="w", name="dvp")
                    for c in range(NCH):
                        py = ps_w.tile([128, 512], FP32, tag="w", name="py")
                        nc.tensor.matmul(py[:],
                                         dyl_sb[32 + 32 * hl:64 + 32 * hl, c * 128:(c + 1) * 128],
                                         aT_sb[32 + 32 * hl:64 + 32 * hl, tm],
                                         start=True, stop=True)
                        rl = work.tile([128, 512], BF16, tag="rl")
                        nc.scalar.activation(rl[:], py[:], AF.Relu)
                        nc.vector.tensor_tensor(
                            xr_sb[:, c * T + m * 512: c * T + (m + 1) * 512], rl[:],
                            x_sb[:, c * T + m * 512: c * T + (m + 1) * 512], AL.mult)
                        nc.tensor.matmul(
                            dvp[:],
                            encl_sb[:, (hl * NCH + c) * D:(hl * NCH + c + 1) * D],
                            xr_sb[:, c * T + m * 512: c * T + (m + 1) * 512],
                            start=(c == 0), stop=(c == NCH - 1),
                            skip_group_check=True)
                    if hl == 0:
                        nc.vector.tensor_copy(dv_sb[:, tm], dvp[:])
                    else:
                        nc.vector.tensor_tensor(dv_sb[:, tm], dv_sb[:, tm], dvp[:], AL.add)

            # ---------------- pair all-reduce of dv; v += dv_tot -------------
            inb = dramp.tile([128, T], FP32, tag="inb")
            outb = dramp.tile([128, T], FP32, tag="outb")
            nc.gpsimd.dma_start(inb[:], dv_sb[:])
            if use_collective:
                nc.gpsimd.collective_compute(
                    "AllReduce", AL.add,
                    replica_groups=[[0, 1], [2, 3], [4, 5], [6, 7]],
                    ins=[inb[:].opt()], outs=[outb[:].opt()])
            rb = outb if use_collective else inb
            nc.gpsimd.dma_start(dv2_sb[:], rb[:])

            # ---------------- v += dv; ln2: v = v + ln(v) ----------------
            for m in range(NM):
                tm = slice(m * 512, (m + 1) * 512)
                nc.vector.tensor_tensor(vT[:, tm], vT[:, tm], dv2_sb[:, tm], AL.add)
                rs_b, mr_b = ln_pass(m)
                t0 = work.tile([128, 512], FP32, tag="lnt")
                nc.vector.tensor_tensor(t0[:], vT[:, tm], rs_b[:], AL.mult)
                t1 = work.tile([128, 512], FP32, tag="lnt2")
                nc.vector.tensor_tensor(t1[:], t0[:], mr_b[:], AL.subtract)
                if apply_g2b2:
                    nc.vector.tensor_scalar_mul(t1[:], t1[:], g2f_sb[:])
                    nc.vector.tensor_scalar(t1[:], t1[:], b2f_sb[:], AL.add)
                nc.vector.tensor_tensor(vT[:, tm], vT[:, tm], t1[:], AL.add)

        # ---------------- logits = v @ readout^T ----------------
        for m in range(NM):
            tm = slice(m * 512, (m + 1) * 512)
            nc.vector.tensor_copy(vb_sb[:, tm], vT[:, tm])
        for tt in range(NT):
            pl = ps_w.tile([128, VOCAB], FP32, tag="w", name="pl")
            nc.tensor.matmul(pl[:], vb_sb[:, tt * 128:(tt + 1) * 128], rwt_sb[:],
                             start=True, stop=True)
            lf = work.tile([128, VOCAB], FP32, tag="lf")
            nc.vector.tensor_copy(lf[:], pl[:])
            nc.sync.dma_start(logits_e[tt * 128:(tt + 1) * 128, :], lf[:])

    nc.compile()
    return nc


# ---------------------------------------------------------------------------
# host-side preparation
# ---------------------------------------------------------------------------

def _prep_core_inputs(inputs, core, T):
    b = min(core // 2, np.asarray(inputs["idx"]).shape[0] - 1)
    heads = [0, 1] if core % 2 == 0 else [2, 3]

    idx = np.asarray(inputs["idx"])
    wte = np.asarray(inputs["wte"], np.float32)
    encoder = np.asarray(inputs["encoder"], np.float32)
    decoder_x = np.asarray(inputs["decoder_x"], np.float32)
    decoder_y = np.asarray(inputs["decoder_y"], np.float32)
    readout_w = np.asarray(inputs["readout_w"], np.float32)

    perm = np.concatenate([np.arange(0, NH, 2), np.arange(1, NH, 2)])

    v0T = wte[idx[b, :T]].astype(np.float32).T                 # [D, T]

    dxh = [decoder_x[h][:, perm].astype(BF) for h in heads]    # [32,1024] each
    dyh = [decoder_y[h][:, perm].astype(BF) for h in heads]
    dxl2 = np.concatenate([dxh[0], dxh[1], dxh[0], dxh[1]], 0)  # [128,1024]
    dyl = np.concatenate([np.zeros((32, NH), BF), dyh[0], dyh[1]], 0)  # [96,1024]

    encl = np.zeros((128, 2 * NCH * D), BF)
    encr = encoder.reshape(H, NH, D)
    for hl, h in enumerate(heads):
        ehp = encr[h][perm, :]                                  # [NH, D]
        for c in range(NCH):
            encl[:, (hl * NCH + c) * D:(hl * NCH + c + 1) * D] = \
                ehp[c * 128:(c + 1) * 128, :].astype(BF)

    div = np.exp(np.arange(0, NH, 2, dtype=np.float64) * (-np.log(10000.0) / NH))
    THI = T // 64
    CPW = 2 * THI + 128
    trig = np.zeros((128, 4 * CPW), np.float32)
    thi = np.arange(THI, dtype=np.float64) * 64.0
    tlo = np.arange(64, dtype=np.float64)
    for cp in range(4):
        dk = div[cp * 128:(cp + 1) * 128][:, None]              # [128,1]
        o = cp * CPW
        trig[:, o:o + THI] = np.sin(dk * thi)
        trig[:, o + THI:o + 2 * THI] = np.cos(dk * thi)
        trig[:, o + 2 * THI:o + 2 * THI + 64] = np.sin(dk * tlo)
        trig[:, o + 2 * THI + 64:o + CPW] = np.cos(dk * tlo)

    triu = np.triu(np.ones((128, 128), np.float32), 0).astype(BF)  # keep t>=s
    ident = np.eye(128, dtype=np.float32).astype(BF)
    rwt = readout_w.T.astype(BF)                                # [128, 256]
    sel = np.zeros((128, 128), np.float32)
    for j in range(128):
        hl = (j // 32) % 2
        sel[heads[hl] * DH + (j % 32), j] = 1.0
    sel = sel.astype(BF)

    g1 = np.asarray(inputs["ln1_g"], np.float32); b1 = np.asarray(inputs["ln1_b"], np.float32)
    g2 = np.asarray(inputs["ln2_g"], np.float32); b2 = np.asarray(inputs["ln2_b"], np.float32)
    a1 = not (np.all(g1 == 1.0) and np.all(b1 == 0.0))
    a2 = not (np.all(g2 == 1.0) and np.all(b2 == 0.0))

    offs = _blob_offsets(T, a1, a2)
    blob = np.zeros((128, offs["_total"]), np.float32)

    def put32(name, arr, rows=slice(0, 128)):
        o, w = offs[name]
        blob[rows, o:o + w] = arr
    def putbf(name, arr_bf, rows=slice(0, 128)):
        o, w = offs[name]
        blob[rows, o:o + arr_bf.shape[1] // 2] = \
            np.ascontiguousarray(arr_bf).view(np.float32)

    putbf("v0T", v0T.astype(BF))
    putbf("dxl2", dxl2)
    putbf("dyl", dyl, rows=slice(0, 96))
    putbf("encl", encl)
    put32("trig", trig)
    putbf("triu", triu)
    putbf("ident", ident)
    putbf("rwt", rwt)
    putbf("sel", sel)
    if a1:
        g1c = np.array([g1[heads[(j // 32) % 2] * DH + (j % 32)] for j in range(128)])
        b1c = np.array([b1[heads[(j // 32) % 2] * DH + (j % 32)] for j in range(128)])
        put32("g1c", g1c[:, None])
        put32("b1c", b1c[:, None])
    if a2:
        put32("g2f", g2[:, None])
        put32("b2f", b2[:, None])
    return {"blob": blob}


_BUILT = {}


def _get_kernel(T, apply_g1b1, apply_g2b2):
    key = (T, apply_g1b1, apply_g2b2)
    if key not in _BUILT:
        _BUILT[key] = build_kernel(T, apply_g1b1, apply_g2b2)
    return _BUILT[key]


def kernel(**inputs) -> np.ndarray:
    idx = np.asarray(inputs["idx"])
    B, T = idx.shape
    g1 = np.asarray(inputs["ln1_g"], np.float32); b1 = np.asarray(inputs["ln1_b"], np.float32)
    g2 = np.asarray(inputs["ln2_g"], np.float32); b2 = np.asarray(inputs["ln2_b"], np.float32)
    a1 = not (np.all(g1 == 1.0) and np.all(b1 == 0.0))
    a2 = not (np.all(g2 == 1.0) and np.all(b2 == 0.0))

    nc = _get_kernel(T, a1, a2)
    in_maps = [_prep_core_inputs(inputs, c, T) for c in range(NCORES)]
    res = run_bass_kernel_spmd(nc, in_maps, list(range(NCORES)))
    out = np.stack([res.results[2 * b]["logits"] for b in range(B)], 0)
    return out.astype(np.float32)
